# revision 41
# baseline (speedup 1.0000x reference)
"""Distributed causal multi-head attention for 8 TRN2 NeuronCores.

Problem: x[4,2048,1024] -> qkv proj -> 16-head causal attention -> out proj.
Sharding (Megatron TP over heads x DP over batch): core i handles batch
b=i//2 and head group g=i%2 (8 heads of 64 dims each). qkv weights are
column-sliced, proj weights row-sliced; the two cores of a batch pair
combine their partial projection outputs with an on-device pairwise
ReduceScatter, so core 2b returns tokens 0..1023 and core 2b+1 tokens
1024..2047 of batch b.

Per-core compute (all matmuls bf16, fp32 PSUM accumulation):
  QK^T = w_qk^T @ x^T  (features on partitions -> Q^T,K^T layout [dh, t])
  V    = x @ w_v       (tokens on partitions, with a ones-column appended)
  S^T  = K^T.T @ Q^T   per head/kv-chunk -> exp -> causal mask-mult
  O^T_aug = V_aug.T @ P^T  (row 64 = softmax denominators, ones-col trick)
  O^T  = O^T_aug * (1/denom) (DMA partition-broadcast of the recip row)
  part = O^T.T @ pw (+ proj bias on even cores) -> ReduceScatter(pair)
"""
import functools
import os
import sys
import types

sys.path.insert(0, "/opt/trn_rl_repo")

import numpy as np
import ml_dtypes

# ---------------------------------------------------------------------------
# antenv.axon_hooks shim: enables NTFF profiling (trace=True) under axon.
# Harmless when tracing is off.
# ---------------------------------------------------------------------------
def _install_ntff_shim():
    try:
        import antenv
    except ImportError:
        return
    if "antenv.axon_hooks" in sys.modules:
        return
    mod = types.ModuleType("antenv.axon_hooks")
    mod._hook = None
    def set_axon_ntff_profile_hook(h):
        mod._hook = h
    def get_axon_ntff_profile_hook():
        return mod._hook
    mod.set_axon_ntff_profile_hook = set_axon_ntff_profile_hook
    mod.get_axon_ntff_profile_hook = get_axon_ntff_profile_hook
    sys.modules["antenv.axon_hooks"] = mod
    antenv.axon_hooks = mod
    try:
        from trn_agent_boot.trn_boot import _ntff_profile_via_ctypes
        hook = _ntff_profile_via_ctypes("/opt/axon/libaxon_pjrt.so")
        if hook is not None:
            set_axon_ntff_profile_hook(hook)
    except Exception:
        pass


_install_ntff_shim()

import concourse.bass as bass
import concourse.mybir as mybir
import concourse.tile as tile
from concourse import bacc
from concourse import bass_utils

bass_utils.upload_artifacts = lambda tmpdir: "local://skipped"
from concourse.bass_utils import run_bass_kernel_spmd

BF16 = mybir.dt.bfloat16
F32 = mybir.dt.float32
NP_BF16 = ml_dtypes.bfloat16

B, T, C = 4, 2048, 1024
N_HEADS_LOCAL = 8          # heads per core
DH = 64
HD = N_HEADS_LOCAL * DH    # 512 local head dims
N_CORES = 8
GROUPS = [[0, 1], [2, 3], [4, 5], [6, 7]]
QT = 512                   # q tile (free dim)
KC = 128                   # kv chunk (partitions)
N_QT = T // QT             # 4
N_KC = T // KC             # 16
VW = DH + 1                # v tile width per head incl. ones column


def build_graph():
    nc = bacc.Bacc("TRN2", target_bir_lowering=False, debug=False,
                   enable_asserts=True, num_devices=N_CORES)

    xT_ext = nc.dram_tensor("xT", [C, T], BF16, kind="ExternalInput").ap()
    wqk_ext = nc.dram_tensor("w_qk", [C, 2 * HD], BF16, kind="ExternalInput").ap()
    wv_ext = nc.dram_tensor("w_v", [C, HD], BF16, kind="ExternalInput").ap()
    bqk_ext = nc.dram_tensor("b_qk", [128, 8], F32, kind="ExternalInput").ap()
    vb_ext = nc.dram_tensor("vb", [128, HD], F32, kind="ExternalInput").ap()
    pw_ext = nc.dram_tensor("pw", [HD, C], BF16, kind="ExternalInput").ap()
    pb_ext = nc.dram_tensor("pb", [128, C], F32, kind="ExternalInput").ap()
    mask_ext = nc.dram_tensor("masks", [4, KC, QT], BF16, kind="ExternalInput").ap()
    out_ext = nc.dram_tensor("out", [T // 2, C], BF16, kind="ExternalOutput").ap()

    partial_dram = nc.dram_tensor("partial", [T, C], BF16).ap()
    rs_dram = nc.dram_tensor("rs_out", [T // 2, C], BF16).ap()

    with tile.TileContext(nc) as tc:
        with (
            tc.tile_pool(name="persist", bufs=1) as persist,
            tc.tile_pool(name="xtp", bufs=2) as xtp,
            tc.tile_pool(name="pt_pool", bufs=32) as pt_pool,
            tc.tile_pool(name="small", bufs=3) as small,
            tc.tile_pool(name="outp", bufs=2) as outp,
            tc.tile_pool(name="ps_s", bufs=4, space="PSUM") as ps_s,
            tc.tile_pool(name="ps_acc", bufs=2, space="PSUM") as ps_acc,
            tc.tile_pool(name="ps_o", bufs=2, space="PSUM") as ps_o,
        ):
            # ---- PE warm-up: dependency-free matmuls on scratch data ------
            # The HAM clock-gate needs ~3.4us of sustained PE activity to
            # lift the PE from 1.2 to 2.4 GHz. These run while the first
            # input DMAs are still in flight, so the real matmuls start warm.
            warm_sb = persist.tile([128, 640], BF16, tag="warm")
            nc.vector.memset(warm_sb[:], 0.0)
            warm_ps = ps_acc.tile([128, QT], F32, tag="acc128")
            for _ in range(40):
                nc.tensor.matmul(warm_ps[:], warm_sb[:, 0:128],
                                 warm_sb[:, 128:640], start=True, stop=True)

            # ---- resident weights / constants -----------------------------
            wqk_sb = persist.tile([128, 8, 2 * HD], BF16, tag="wqk")
            wqk_r = wqk_ext.rearrange("(o p) n -> p o n", p=128)
            for cc in range(8):
                nc.sync.dma_start(wqk_sb[:, cc:cc + 1, :], wqk_r[:, cc:cc + 1, :])
            bqk_sb = persist.tile([128, 8], F32, tag="bqk")
            nc.sync.dma_start(bqk_sb[:], bqk_ext[:])
            xT_r = xT_ext.rearrange("(o p) t -> p o t", p=128)
            xs0 = xtp.tile([128, 8, QT], BF16, tag="xt", name="xt0")
            nc.sync.dma_start(xs0[:], xT_r[:, :, 0:QT])
            wv_sb = persist.tile([128, 8, HD], BF16, tag="wv")
            wv_r = wv_ext.rearrange("(o p) n -> p o n", p=128)
            for cc in range(0, 8, 2):
                nc.sync.dma_start(wv_sb[:, cc:cc + 2, :], wv_r[:, cc:cc + 2, :])
            vb_sb = persist.tile([128, HD], F32, tag="vb")
            nc.sync.dma_start(vb_sb[:], vb_ext[:])
            mask_sb = persist.tile([128, 4, QT], BF16, tag="mask")
            nc.sync.dma_start(mask_sb[:], mask_ext.rearrange("v p q -> p v q"))
            pw_sb = persist.tile([128, 4, C], BF16, tag="pw")
            nc.sync.dma_start(pw_sb[:], pw_ext.rearrange("(o p) n -> p o n", p=128))
            pb_sb = persist.tile([128, C], F32, tag="pb")
            nc.sync.dma_start(pb_sb[:], pb_ext[:])


            # persistent activations
            qkt = [persist.tile([128, T], BF16, tag=f"qkt{o}", name=f"qkt{o}")
                   for o in range(8)]
            vt = [persist.tile([128, N_HEADS_LOCAL, VW], BF16, tag=f"v{kc}",
                               name=f"v{kc}") for kc in range(N_KC)]
            ot = [persist.tile([128, T], BF16, tag=f"ot{hc}", name=f"ot{hc}")
                  for hc in range(4)]

            xs_tiles = {}

            def emit_x_dma(tq):
                if tq == 0:
                    xs_tiles[0] = xs0
                else:
                    xs = xtp.tile([128, 8, QT], BF16, tag="xt", name=f"xt{tq}")
                    nc.sync.dma_start(xs[:], xT_r[:, :, tq * QT:(tq + 1) * QT])
                    xs_tiles[tq] = xs

            def qkv_parts(tq):
                """12 closures: 8 QK^T feature chunks + 4 V kv-chunks."""
                tsl = slice(tq * QT, (tq + 1) * QT)

                def qk_part(o):
                    xs = xs_tiles[tq]
                    ps = ps_acc.tile([128, QT], F32, tag="acc128")
                    for cc in range(8):
                        nc.tensor.matmul(
                            ps[:],
                            wqk_sb[:, cc, o * 128:(o + 1) * 128],
                            xs[:, cc, :],
                            start=(cc == 0), stop=(cc == 7),
                        )
                    nc.vector.tensor_add(
                        qkt[o][:, tsl], ps[:],
                        bqk_sb[:, o:o + 1].to_broadcast((128, QT)),
                    )

                def v_part(kc):
                    xs = xs_tiles[tq]
                    ks = kc * KC - tq * QT
                    ps = ps_acc.tile([128, HD], F32, tag="acc128")
                    for cc in range(8):
                        nc.tensor.matmul(
                            ps[:],
                            xs[:, cc, ks:ks + KC],
                            wv_sb[:, cc, :],
                            start=(cc == 0), stop=(cc == 7),
                        )
                    nc.vector.tensor_add(
                        vt[kc][:, :, 0:DH],
                        ps[:].rearrange("p (h d) -> p h d", h=N_HEADS_LOCAL),
                        vb_sb[:].rearrange("p (h d) -> p h d", h=N_HEADS_LOCAL),
                    )
                    nc.vector.memset(vt[kc][:, :, DH:VW], 1.0)

                return ([functools.partial(qk_part, o) for o in range(8)],
                        [functools.partial(v_part, kc)
                         for kc in range(4 * tq, 4 * tq + 4)])

            def emit_attention_group(qt, hc, filler=None, fill_every=5,
                                     filler_fast=None):
                """Scores/exp/mask + AV (software-pipelined), one head pair.
                Pulls one PE-dense filler part every fill_every kc steps to
                keep the PE warm while ACT works through the exps."""
                nkc = 4 * qt + 4
                qsl = slice(qt * QT, (qt + 1) * QT)
                LAG = 2
                if True:
                    pts = {0: [], 1: []}
                    po = {0: ps_o.tile([VW, QT], F32, tag="o", name="poA"),
                          1: ps_o.tile([VW, QT], F32, tag="o", name="poB")}
                    for step in range(nkc + LAG):
                        if step < nkc:
                            kc = step
                            ksl = slice(kc * KC, (kc + 1) * KC)
                            v = kc - 4 * qt
                            for sub in range(2):
                                hp = sub * 64
                                ps = ps_s.tile([128, QT], F32, tag="s")
                                nc.tensor.matmul(
                                    ps[:],
                                    qkt[4 + hc][hp:hp + 64, ksl],
                                    qkt[hc][hp:hp + 64, qsl],
                                    start=True, stop=True,
                                    tile_position=(hp, 0),
                                )
                                pt = pt_pool.tile([128, QT], BF16, tag="pt")
                                if v >= 1:
                                    # left 128*v cols fully causal-masked
                                    z = 128 * v
                                    nc.vector.memset(pt[:, 0:z], 0.0)
                                    nc.scalar.activation(
                                        pt[:, z:QT], ps[:, z:QT],
                                        mybir.ActivationFunctionType.Exp)
                                    nc.vector.tensor_mul(
                                        pt[:, z:QT], pt[:, z:QT],
                                        mask_sb[:, v, z:QT])
                                else:
                                    nc.scalar.activation(
                                        pt[:], ps[:],
                                        mybir.ActivationFunctionType.Exp)
                                    if v == 0:
                                        nc.vector.tensor_mul(
                                            pt[:], pt[:], mask_sb[:, 0, :])
                                pts[sub].append(pt)
                        av_k = step - LAG
                        if av_k >= 0:
                            for sub in range(2):
                                h = 2 * hc + sub
                                nc.tensor.matmul(
                                    po[sub][:], vt[av_k][:, h, :],
                                    pts[sub][av_k][:],
                                    start=(av_k == 0), stop=(av_k == nkc - 1),
                                )
                        if filler_fast is not None and step % 2 == 1:
                            part = next(filler_fast, None)
                            if part is not None:
                                part()
                                continue_slow = False
                            else:
                                continue_slow = True
                        else:
                            continue_slow = True
                        if (continue_slow and filler is not None
                                and step % fill_every == fill_every - 1):
                            part = next(filler, None)
                            if part is not None:
                                part()
                    for sub in range(2):
                        hp = sub * 64
                        den = small.tile([1, QT], F32, tag="den")
                        nc.vector.tensor_copy(den[:], po[sub][DH:VW, :])
                        recip = small.tile([1, QT], F32, tag="recip")
                        nc.vector.reciprocal_approx_fast(recip[:], den[:])
                        bcast = small.tile([64, QT], F32, tag="bcast")
                        nc.gpsimd.partition_broadcast(bcast[:], recip[:])
                        nc.vector.tensor_mul(
                            ot[hc][hp:hp + 64, qsl],
                            po[sub][0:DH, :], bcast[:],
                        )

            def emit_rs(tok_start, ntok):
                # rows tok_start//2.. of rs_dram receive this core's half
                nc.gpsimd.collective_compute(
                    "ReduceScatter",
                    mybir.AluOpType.add,
                    replica_groups=GROUPS,
                    ins=[partial_dram[tok_start:tok_start + ntok, :]],
                    outs=[rs_dram[tok_start // 2:(tok_start + ntok) // 2, :]],
                )

            def emit_cast_out(row_start, nrows):
                # bf16 all the way out; host upcasts. Pure dram-to-dram DMA,
                # touches no compute engine.
                nc.sync.dma_start(
                    out_ext[row_start:row_start + nrows, :],
                    rs_dram[row_start:row_start + nrows, :])

            def proj_part(qt, tsub):
                tt = qt * (QT // KC) + tsub
                for ct in range(2):
                    ps = ps_acc.tile([128, QT], F32, tag="acc128")
                    for hc in range(4):
                        nc.tensor.matmul(
                            ps[:],
                            ot[hc][:, tt * KC:(tt + 1) * KC],
                            pw_sb[:, hc, ct * QT:(ct + 1) * QT],
                            start=(hc == 0), stop=(hc == 3),
                        )
                    po = outp.tile([128, QT], BF16, tag="po")
                    nc.vector.tensor_add(po[:], ps[:],
                                         pb_sb[:, ct * QT:(ct + 1) * QT])
                    nc.sync.dma_start(
                        partial_dram[tt * KC:(tt + 1) * KC,
                                     ct * QT:(ct + 1) * QT],
                        po[:],
                    )

            # ---- woven software-pipelined schedule ------------------------
            # Between the 4 ACT-bound attention head-groups of block qt we
            # weave PE-dense filler: QKV parts of qt+1, projection parts of
            # qt-1, its RS, and the (safe, long-completed) cast of qt-2.
            emit_x_dma(0)
            qk0, v0 = qkv_parts(0)
            for part in qk0 + v0:
                part()
            v3_parts = []
            for qt in range(N_QT):
                queue = []
                if qt + 1 < N_QT:
                    emit_x_dma(qt + 1)
                    qk, v = qkv_parts(qt + 1)
                    if qt + 1 == N_QT - 1:
                        # block 3's V chunks feed qt3's starving groups; its
                        # AV steps need them only from step 15 of group 0,
                        # and the fill cadence below lands them by step 11
                        queue += qk
                        v3_parts = v
                    else:
                        queue += qk + v
                if qt == N_QT - 1:
                    queue = v3_parts + queue
                if qt >= 1:
                    queue += [functools.partial(proj_part, qt - 1, ts)
                              for ts in range(4)]
                    queue.append(functools.partial(emit_rs, (qt - 1) * QT, QT))
                if qt >= 2:
                    queue.append(functools.partial(
                        emit_cast_out, (qt - 2) * 256, 256))
                filler = iter(queue)
                steps = 4 * (4 * qt + 4 + 3)
                if qt == N_QT - 1:
                    fill_every = 3
                else:
                    fill_every = max(2, steps // max(len(queue), 1))
                for hc in range(4):
                    emit_attention_group(qt, hc, filler, fill_every)
                for part in filler:
                    part()
            # drain
            emit_cast_out((N_QT - 2) * 256, 256)
            for half in range(2):
                for ts in (0, 1) if half == 0 else (2, 3):
                    proj_part(N_QT - 1, ts)
                emit_rs((N_QT - 1) * QT + half * 256, 256)
                emit_cast_out((N_QT - 1) * 256 + half * 128, 128)

    nc.compile()
    return nc


_NC = None


def _get_nc():
    global _NC
    if _NC is None:
        _NC = build_graph()
    return _NC


def make_masks():
    # masks[v][k_l, q_l] = 1 where (128*v + k_l) <= q_l else 0
    v = np.arange(4)[:, None, None]
    k = np.arange(KC)[None, :, None]
    q = np.arange(QT)[None, None, :]
    return ((128 * v + k) <= q).astype(np.float32)


def shard_inputs(x, qkv_w, qkv_b, proj_w, proj_b):
    x = np.asarray(x, np.float32)
    qkv_w = np.asarray(qkv_w, np.float32)
    qkv_b = np.asarray(qkv_b, np.float32)
    proj_w = np.asarray(proj_w, np.float32)
    proj_b = np.asarray(proj_b, np.float32)
    scale = DH ** (-0.5)
    masks = make_masks().astype(NP_BF16)
    in_maps = []
    for i in range(N_CORES):
        b, g = i // 2, i % 2
        sl = slice(g * HD, (g + 1) * HD)
        w_qk = np.concatenate(
            [qkv_w[:, 0:C][:, sl] * scale, qkv_w[:, C:2 * C][:, sl]], axis=1)
        b_cat = np.concatenate(
            [qkv_b[0:C][sl] * scale, qkv_b[C:2 * C][sl]])
        in_maps.append({
            "xT": np.ascontiguousarray(x[b].T).astype(NP_BF16),
            "w_qk": w_qk.astype(NP_BF16),
            "w_v": qkv_w[:, 2 * C:3 * C][:, sl].astype(NP_BF16),
            "b_qk": np.ascontiguousarray(b_cat.reshape(8, 128).T),
            "vb": np.tile(qkv_b[2 * C:3 * C][sl][None, :], (128, 1)),
            "pw": proj_w[sl, :].astype(NP_BF16),
            "pb": (np.tile(proj_b[None, :], (128, 1)) if g == 0
                   else np.zeros((128, C), np.float32)),
            "masks": masks,
        })
    return in_maps


LAST_RESULT = None


def kernel(x, qkv_w, qkv_b, proj_w, proj_b):
    global LAST_RESULT
    nc = _get_nc()
    in_maps = shard_inputs(x, qkv_w, qkv_b, proj_w, proj_b)
    trace = bool(int(os.environ.get("BASS_KERNEL_TRACE", "0")))
    kwargs = {}
    if trace:
        kwargs = dict(trace=True, trace_cores=list(range(N_CORES)))
    res = run_bass_kernel_spmd(nc, in_maps, core_ids=list(range(N_CORES)), **kwargs)
    LAST_RESULT = res
    out = np.empty((B, T, C), np.float32)
    for i in range(N_CORES):
        b, g = i // 2, i % 2
        o = np.asarray(res.results[i]["out"]).astype(np.float32)
        for qt in range(N_QT - 1):
            out[b, qt * QT + g * 256: qt * QT + (g + 1) * 256, :] = \
                o[qt * 256:(qt + 1) * 256]
        for half in range(2):
            start = (N_QT - 1) * QT + half * 256
            out[b, start + g * 128: start + (g + 1) * 128, :] = \
                o[start // 2:start // 2 + 128]
    return out


# revision 43
# speedup vs baseline: 1.0212x; 1.0212x over previous
"""Distributed causal multi-head attention for 8 TRN2 NeuronCores.

Problem: x[4,2048,1024] -> qkv proj -> 16-head causal attention -> out proj.
Sharding (Megatron TP over heads x DP over batch): core i handles batch
b=i//2 and head group g=i%2 (8 heads of 64 dims each). qkv weights are
column-sliced, proj weights row-sliced; the two cores of a batch pair
combine their partial projection outputs with an on-device pairwise
ReduceScatter, so core 2b returns tokens 0..1023 and core 2b+1 tokens
1024..2047 of batch b.

Per-core compute (all matmuls bf16, fp32 PSUM accumulation):
  QK^T = w_qk^T @ x^T  (features on partitions -> Q^T,K^T layout [dh, t])
  V    = x @ w_v       (tokens on partitions, with a ones-column appended)
  S^T  = K^T.T @ Q^T   per head/kv-chunk -> exp -> causal mask-mult
  O^T_aug = V_aug.T @ P^T  (row 64 = softmax denominators, ones-col trick)
  O^T  = O^T_aug * (1/denom) (DMA partition-broadcast of the recip row)
  part = O^T.T @ pw (+ proj bias on even cores) -> ReduceScatter(pair)
"""
import functools
import os
import sys
import types

sys.path.insert(0, "/opt/trn_rl_repo")

import numpy as np
import ml_dtypes

# ---------------------------------------------------------------------------
# antenv.axon_hooks shim: enables NTFF profiling (trace=True) under axon.
# Harmless when tracing is off.
# ---------------------------------------------------------------------------
def _install_ntff_shim():
    try:
        import antenv
    except ImportError:
        return
    if "antenv.axon_hooks" in sys.modules:
        return
    mod = types.ModuleType("antenv.axon_hooks")
    mod._hook = None
    def set_axon_ntff_profile_hook(h):
        mod._hook = h
    def get_axon_ntff_profile_hook():
        return mod._hook
    mod.set_axon_ntff_profile_hook = set_axon_ntff_profile_hook
    mod.get_axon_ntff_profile_hook = get_axon_ntff_profile_hook
    sys.modules["antenv.axon_hooks"] = mod
    antenv.axon_hooks = mod
    try:
        from trn_agent_boot.trn_boot import _ntff_profile_via_ctypes
        hook = _ntff_profile_via_ctypes("/opt/axon/libaxon_pjrt.so")
        if hook is not None:
            set_axon_ntff_profile_hook(hook)
    except Exception:
        pass


_install_ntff_shim()

import concourse.bass as bass
import concourse.mybir as mybir
import concourse.tile as tile
from concourse import bacc
from concourse import bass_utils

bass_utils.upload_artifacts = lambda tmpdir: "local://skipped"
from concourse.bass_utils import run_bass_kernel_spmd

BF16 = mybir.dt.bfloat16
F32 = mybir.dt.float32
NP_BF16 = ml_dtypes.bfloat16

B, T, C = 4, 2048, 1024
N_HEADS_LOCAL = 8          # heads per core
DH = 64
HD = N_HEADS_LOCAL * DH    # 512 local head dims
N_CORES = 8
GROUPS = [[0, 1], [2, 3], [4, 5], [6, 7]]
QT = 512                   # q tile (free dim)
KC = 128                   # kv chunk (partitions)
N_QT = T // QT             # 4
N_KC = T // KC             # 16
VW = DH + 1                # v tile width per head incl. ones column


def build_graph():
    nc = bacc.Bacc("TRN2", target_bir_lowering=False, debug=False,
                   enable_asserts=True, num_devices=N_CORES)

    xT_ext = nc.dram_tensor("xT", [C, T], BF16, kind="ExternalInput").ap()
    wqk_ext = nc.dram_tensor("w_qk", [C, 2 * HD], BF16, kind="ExternalInput").ap()
    wv_ext = nc.dram_tensor("w_v", [C, HD], BF16, kind="ExternalInput").ap()
    bqk_ext = nc.dram_tensor("b_qk", [128, 8], F32, kind="ExternalInput").ap()
    vb_ext = nc.dram_tensor("vb", [128, HD], F32, kind="ExternalInput").ap()
    pw_ext = nc.dram_tensor("pw", [HD, C], BF16, kind="ExternalInput").ap()
    pb_ext = nc.dram_tensor("pb", [128, C], F32, kind="ExternalInput").ap()
    mask_ext = nc.dram_tensor("masks", [4, KC, QT], BF16, kind="ExternalInput").ap()
    out_ext = nc.dram_tensor("out", [T // 2, C], BF16, kind="ExternalOutput").ap()

    partial_dram = nc.dram_tensor("partial", [T, C], BF16).ap()
    rs_dram = nc.dram_tensor("rs_out", [T // 2, C], BF16).ap()

    with tile.TileContext(nc) as tc:
        with (
            tc.tile_pool(name="persist", bufs=1) as persist,
            tc.tile_pool(name="xtp", bufs=2) as xtp,
            tc.tile_pool(name="pt_pool", bufs=40) as pt_pool,
            tc.tile_pool(name="small", bufs=3) as small,
            tc.tile_pool(name="outp", bufs=2) as outp,
            tc.tile_pool(name="ps_s", bufs=3, space="PSUM") as ps_s,
            tc.tile_pool(name="ps_acc", bufs=2, space="PSUM") as ps_acc,
            tc.tile_pool(name="ps_o", bufs=3, space="PSUM") as ps_o,
        ):
            # ---- PE warm-up: dependency-free matmuls on scratch data ------
            # The HAM clock-gate needs ~3.4us of sustained PE activity to
            # lift the PE from 1.2 to 2.4 GHz. These run while the first
            # input DMAs are still in flight, so the real matmuls start warm.
            warm_sb = persist.tile([128, 640], BF16, tag="warm")
            nc.vector.memset(warm_sb[:], 0.0)
            warm_ps = ps_acc.tile([128, QT], F32, tag="acc128")
            for _ in range(40):
                nc.tensor.matmul(warm_ps[:], warm_sb[:, 0:128],
                                 warm_sb[:, 128:640], start=True, stop=True)

            # ---- resident weights / constants -----------------------------
            wqk_sb = persist.tile([128, 8, 2 * HD], BF16, tag="wqk")
            wqk_r = wqk_ext.rearrange("(o p) n -> p o n", p=128)
            for cc in range(8):
                nc.sync.dma_start(wqk_sb[:, cc:cc + 1, :], wqk_r[:, cc:cc + 1, :])
            bqk_sb = persist.tile([128, 8], F32, tag="bqk")
            nc.sync.dma_start(bqk_sb[:], bqk_ext[:])
            xT_r = xT_ext.rearrange("(o p) t -> p o t", p=128)
            xs0 = xtp.tile([128, 8, QT], BF16, tag="xt", name="xt0")
            nc.sync.dma_start(xs0[:], xT_r[:, :, 0:QT])
            wv_sb = persist.tile([128, 8, HD], BF16, tag="wv")
            wv_r = wv_ext.rearrange("(o p) n -> p o n", p=128)
            for cc in range(0, 8, 2):
                nc.sync.dma_start(wv_sb[:, cc:cc + 2, :], wv_r[:, cc:cc + 2, :])
            vb_sb = persist.tile([128, HD], F32, tag="vb")
            nc.sync.dma_start(vb_sb[:], vb_ext[:])
            mask_sb = persist.tile([128, 4, QT], BF16, tag="mask")
            nc.sync.dma_start(mask_sb[:], mask_ext.rearrange("v p q -> p v q"))
            pw_sb = persist.tile([128, 4, C], BF16, tag="pw")
            nc.sync.dma_start(pw_sb[:], pw_ext.rearrange("(o p) n -> p o n", p=128))
            pb_sb = persist.tile([128, C], F32, tag="pb")
            nc.sync.dma_start(pb_sb[:], pb_ext[:])


            # persistent activations
            qkt = [persist.tile([128, T], BF16, tag=f"qkt{o}", name=f"qkt{o}")
                   for o in range(8)]
            vt = [persist.tile([128, N_HEADS_LOCAL, VW], BF16, tag=f"v{kc}",
                               name=f"v{kc}") for kc in range(N_KC)]
            ot = [persist.tile([128, T], BF16, tag=f"ot{hc}", name=f"ot{hc}")
                  for hc in range(4)]

            xs_tiles = {}

            def emit_x_dma(tq):
                if tq == 0:
                    xs_tiles[0] = xs0
                else:
                    xs = xtp.tile([128, 8, QT], BF16, tag="xt", name=f"xt{tq}")
                    nc.sync.dma_start(xs[:], xT_r[:, :, tq * QT:(tq + 1) * QT])
                    xs_tiles[tq] = xs

            def qkv_parts(tq):
                """12 closures: 8 QK^T feature chunks + 4 V kv-chunks."""
                tsl = slice(tq * QT, (tq + 1) * QT)

                def qk_part(o):
                    xs = xs_tiles[tq]
                    ps = ps_acc.tile([128, QT], F32, tag="acc128")
                    for cc in range(8):
                        nc.tensor.matmul(
                            ps[:],
                            wqk_sb[:, cc, o * 128:(o + 1) * 128],
                            xs[:, cc, :],
                            start=(cc == 0), stop=(cc == 7),
                        )
                    nc.vector.tensor_add(
                        qkt[o][:, tsl], ps[:],
                        bqk_sb[:, o:o + 1].to_broadcast((128, QT)),
                    )

                def v_part(kc):
                    xs = xs_tiles[tq]
                    ks = kc * KC - tq * QT
                    ps = ps_acc.tile([128, HD], F32, tag="acc128")
                    for cc in range(8):
                        nc.tensor.matmul(
                            ps[:],
                            xs[:, cc, ks:ks + KC],
                            wv_sb[:, cc, :],
                            start=(cc == 0), stop=(cc == 7),
                        )
                    nc.vector.tensor_add(
                        vt[kc][:, :, 0:DH],
                        ps[:].rearrange("p (h d) -> p h d", h=N_HEADS_LOCAL),
                        vb_sb[:].rearrange("p (h d) -> p h d", h=N_HEADS_LOCAL),
                    )
                    nc.vector.memset(vt[kc][:, :, DH:VW], 1.0)

                return ([functools.partial(qk_part, o) for o in range(8)],
                        [functools.partial(v_part, kc)
                         for kc in range(4 * tq, 4 * tq + 4)])

            def emit_attention_group(qt, hc, filler=None, fill_every=5,
                                     filler_fast=None):
                """Scores/exp/mask + AV (software-pipelined), one head pair.
                Pulls one PE-dense filler part every fill_every kc steps to
                keep the PE warm while ACT works through the exps."""
                nkc = 4 * qt + 4
                qsl = slice(qt * QT, (qt + 1) * QT)
                LAG = 3
                if True:
                    pts = {0: [], 1: []}
                    po = {0: ps_o.tile([VW, QT], F32, tag="o", name="poA"),
                          1: ps_o.tile([VW, QT], F32, tag="o", name="poB")}
                    for step in range(nkc + LAG):
                        if step < nkc:
                            kc = step
                            ksl = slice(kc * KC, (kc + 1) * KC)
                            v = kc - 4 * qt
                            for sub in range(2):
                                hp = sub * 64
                                ps = ps_s.tile([128, QT], F32, tag="s")
                                nc.tensor.matmul(
                                    ps[:],
                                    qkt[4 + hc][hp:hp + 64, ksl],
                                    qkt[hc][hp:hp + 64, qsl],
                                    start=True, stop=True,
                                    tile_position=(hp, 0),
                                )
                                pt = pt_pool.tile([128, QT], BF16, tag="pt")
                                if v >= 1:
                                    # left 128*v cols fully causal-masked
                                    z = 128 * v
                                    nc.vector.memset(pt[:, 0:z], 0.0)
                                    nc.scalar.activation(
                                        pt[:, z:QT], ps[:, z:QT],
                                        mybir.ActivationFunctionType.Exp)
                                    nc.vector.tensor_mul(
                                        pt[:, z:QT], pt[:, z:QT],
                                        mask_sb[:, v, z:QT])
                                else:
                                    nc.scalar.activation(
                                        pt[:], ps[:],
                                        mybir.ActivationFunctionType.Exp)
                                    if v == 0:
                                        nc.vector.tensor_mul(
                                            pt[:], pt[:], mask_sb[:, 0, :])
                                pts[sub].append(pt)
                        av_k = step - LAG
                        if av_k >= 0:
                            for sub in range(2):
                                h = 2 * hc + sub
                                nc.tensor.matmul(
                                    po[sub][:], vt[av_k][:, h, :],
                                    pts[sub][av_k][:],
                                    start=(av_k == 0), stop=(av_k == nkc - 1),
                                )
                        if filler_fast is not None and step % 2 == 1:
                            part = next(filler_fast, None)
                            if part is not None:
                                part()
                                continue_slow = False
                            else:
                                continue_slow = True
                        else:
                            continue_slow = True
                        if (continue_slow and filler is not None
                                and step % fill_every == fill_every - 1):
                            part = next(filler, None)
                            if part is not None:
                                part()
                    for sub in range(2):
                        hp = sub * 64
                        den = small.tile([1, QT], F32, tag="den")
                        nc.vector.tensor_copy(den[:], po[sub][DH:VW, :])
                        recip = small.tile([1, QT], F32, tag="recip")
                        nc.vector.reciprocal_approx_fast(recip[:], den[:])
                        bcast = small.tile([64, QT], F32, tag="bcast")
                        nc.gpsimd.partition_broadcast(bcast[:], recip[:])
                        nc.vector.tensor_mul(
                            ot[hc][hp:hp + 64, qsl],
                            po[sub][0:DH, :], bcast[:],
                        )

            def emit_rs(tok_start, ntok):
                # rows tok_start//2.. of rs_dram receive this core's half
                nc.gpsimd.collective_compute(
                    "ReduceScatter",
                    mybir.AluOpType.add,
                    replica_groups=GROUPS,
                    ins=[partial_dram[tok_start:tok_start + ntok, :]],
                    outs=[rs_dram[tok_start // 2:(tok_start + ntok) // 2, :]],
                )

            def emit_cast_out(row_start, nrows):
                # bf16 all the way out; host upcasts. Pure dram-to-dram DMA,
                # touches no compute engine.
                nc.sync.dma_start(
                    out_ext[row_start:row_start + nrows, :],
                    rs_dram[row_start:row_start + nrows, :])

            def proj_part(qt, tsub):
                tt = qt * (QT // KC) + tsub
                for ct in range(2):
                    ps = ps_acc.tile([128, QT], F32, tag="acc128")
                    for hc in range(4):
                        nc.tensor.matmul(
                            ps[:],
                            ot[hc][:, tt * KC:(tt + 1) * KC],
                            pw_sb[:, hc, ct * QT:(ct + 1) * QT],
                            start=(hc == 0), stop=(hc == 3),
                        )
                    po = outp.tile([128, QT], BF16, tag="po")
                    nc.vector.tensor_add(po[:], ps[:],
                                         pb_sb[:, ct * QT:(ct + 1) * QT])
                    nc.sync.dma_start(
                        partial_dram[tt * KC:(tt + 1) * KC,
                                     ct * QT:(ct + 1) * QT],
                        po[:],
                    )

            # ---- woven software-pipelined schedule ------------------------
            # Between the 4 ACT-bound attention head-groups of block qt we
            # weave PE-dense filler: QKV parts of qt+1, projection parts of
            # qt-1, its RS, and the (safe, long-completed) cast of qt-2.
            emit_x_dma(0)
            qk0, v0 = qkv_parts(0)
            for part in qk0 + v0:
                part()
            v3_parts = []
            for qt in range(N_QT):
                queue = []
                if qt + 1 < N_QT:
                    emit_x_dma(qt + 1)
                    qk, v = qkv_parts(qt + 1)
                    if qt + 1 == N_QT - 1:
                        # block 3's V chunks feed qt3's starving groups; its
                        # AV steps need them only from step 15 of group 0,
                        # and the fill cadence below lands them by step 11
                        queue += qk
                        v3_parts = v
                    else:
                        queue += qk + v
                if qt == N_QT - 1:
                    queue = v3_parts + queue
                if qt >= 1:
                    queue += [functools.partial(proj_part, qt - 1, ts)
                              for ts in range(4)]
                    queue.append(functools.partial(emit_rs, (qt - 1) * QT, QT))
                if qt >= 2:
                    queue.append(functools.partial(
                        emit_cast_out, (qt - 2) * 256, 256))
                filler = iter(queue)
                steps = 4 * (4 * qt + 4 + 3)
                if qt == N_QT - 1:
                    fill_every = 3
                else:
                    fill_every = max(2, steps // max(len(queue), 1))
                for hc in range(4):
                    emit_attention_group(qt, hc, filler, fill_every)
                for part in filler:
                    part()
            # drain
            emit_cast_out((N_QT - 2) * 256, 256)
            for half in range(2):
                for ts in (0, 1) if half == 0 else (2, 3):
                    proj_part(N_QT - 1, ts)
                emit_rs((N_QT - 1) * QT + half * 256, 256)
                emit_cast_out((N_QT - 1) * 256 + half * 128, 128)

    nc.compile()
    return nc


_NC = None


def _get_nc():
    global _NC
    if _NC is None:
        _NC = build_graph()
    return _NC


def make_masks():
    # masks[v][k_l, q_l] = 1 where (128*v + k_l) <= q_l else 0
    v = np.arange(4)[:, None, None]
    k = np.arange(KC)[None, :, None]
    q = np.arange(QT)[None, None, :]
    return ((128 * v + k) <= q).astype(np.float32)


def shard_inputs(x, qkv_w, qkv_b, proj_w, proj_b):
    x = np.asarray(x, np.float32)
    qkv_w = np.asarray(qkv_w, np.float32)
    qkv_b = np.asarray(qkv_b, np.float32)
    proj_w = np.asarray(proj_w, np.float32)
    proj_b = np.asarray(proj_b, np.float32)
    scale = DH ** (-0.5)
    masks = make_masks().astype(NP_BF16)
    in_maps = []
    for i in range(N_CORES):
        b, g = i // 2, i % 2
        sl = slice(g * HD, (g + 1) * HD)
        w_qk = np.concatenate(
            [qkv_w[:, 0:C][:, sl] * scale, qkv_w[:, C:2 * C][:, sl]], axis=1)
        b_cat = np.concatenate(
            [qkv_b[0:C][sl] * scale, qkv_b[C:2 * C][sl]])
        in_maps.append({
            "xT": np.ascontiguousarray(x[b].T).astype(NP_BF16),
            "w_qk": w_qk.astype(NP_BF16),
            "w_v": qkv_w[:, 2 * C:3 * C][:, sl].astype(NP_BF16),
            "b_qk": np.ascontiguousarray(b_cat.reshape(8, 128).T),
            "vb": np.tile(qkv_b[2 * C:3 * C][sl][None, :], (128, 1)),
            "pw": proj_w[sl, :].astype(NP_BF16),
            "pb": (np.tile(proj_b[None, :], (128, 1)) if g == 0
                   else np.zeros((128, C), np.float32)),
            "masks": masks,
        })
    return in_maps


LAST_RESULT = None


def kernel(x, qkv_w, qkv_b, proj_w, proj_b):
    global LAST_RESULT
    nc = _get_nc()
    in_maps = shard_inputs(x, qkv_w, qkv_b, proj_w, proj_b)
    trace = bool(int(os.environ.get("BASS_KERNEL_TRACE", "0")))
    kwargs = {}
    if trace:
        kwargs = dict(trace=True, trace_cores=list(range(N_CORES)))
    res = run_bass_kernel_spmd(nc, in_maps, core_ids=list(range(N_CORES)), **kwargs)
    LAST_RESULT = res
    out = np.empty((B, T, C), np.float32)
    for i in range(N_CORES):
        b, g = i // 2, i % 2
        o = np.asarray(res.results[i]["out"]).astype(np.float32)
        for qt in range(N_QT - 1):
            out[b, qt * QT + g * 256: qt * QT + (g + 1) * 256, :] = \
                o[qt * 256:(qt + 1) * 256]
        for half in range(2):
            start = (N_QT - 1) * QT + half * 256
            out[b, start + g * 128: start + (g + 1) * 128, :] = \
                o[start // 2:start // 2 + 128]
    return out
